# revision 1
# baseline (speedup 1.0000x reference)
"""Trainium2 Bass kernel for MissingSensorImputation (GNN message passing).

Strategy (per core, data-parallel over batch: 8 cores x 2 batches):
  - One SWDGE casting DMA reorganizes x into a bf16 HBM table [node, 2*H]
    (512B rows) for gathering.
  - Edges sorted by dst, grouped per 128-node dst block, padded to chunks of
    128 edges. dma_gather pulls each chunk's src rows into SBUF partitions.
  - Scatter-add = PE matmul: onehot(dstlocal)^T @ msgs accumulated in PSUM
    per dst block (onehot generated on DVE via iota==dstlocal compare in
    2x_1p-friendly paired layout).
  - MLP (feature-major): transpose neighbor and x per block on PE,
    h=relu(W1a@nT + W1b@xT + b1), imputed = W2@h + b2, transpose back,
    blend with resident fp32 x by the missing mask.
Host-side preprocessing touches only the edge index (sorting/padding) and
constant tables; all tensor math runs on device.
"""
import sys

sys.path.insert(0, "/opt/trn_rl_repo")

import numpy as np
import concourse.bacc as bacc
import concourse.mybir as mybir
import concourse.tile as tile
from concourse import bass_utils

P = 128
NCORES = 8
F32 = mybir.dt.float32
BF16 = mybir.dt.bfloat16
I16 = mybir.dt.int16
I32 = mybir.dt.int32
NPBF16 = mybir.dt.np(BF16)

AF = mybir.ActivationFunctionType
ALU = mybir.AluOpType

_cache = {}
last_results = None  # BassKernelResults of the most recent run (for profiling)


def _slots(s_b, d_b, N):
    """Group a block's edges by src into slots carrying up to 3 dst columns.

    Returns (src, d1, d2, d3, n2, n3): slot source rows, up to three
    dstlocal values per slot (255 = unused sentinel), and the counts of
    slots having a 2nd / 3rd dst. Slots with more dsts sort first so d2/d3
    occupy a prefix.
    """
    o = np.argsort(s_b, kind="stable")
    sv, dv = s_b[o], d_b[o]
    n = len(sv)
    if n == 0:
        z = np.zeros(0, np.int64)
        return z, z, z, z, 0, 0
    new = np.r_[True, sv[1:] != sv[:-1]]
    runstart = np.maximum.accumulate(np.where(new, np.arange(n), 0))
    pos = np.arange(n) - runstart
    sub = pos // 3
    rank = pos % 3
    runid = np.cumsum(new) - 1
    skey = runid * (n + 1) + sub
    uniq, inv = np.unique(skey, return_inverse=True)
    ns = len(uniq)
    slot_src = np.zeros(ns, np.int64)
    d1 = np.zeros(ns, np.int64)
    d2 = np.full(ns, 255, np.int64)
    d3 = np.full(ns, 255, np.int64)
    slot_src[inv] = sv
    d1[inv[rank == 0]] = dv[rank == 0]
    d2[inv[rank == 1]] = dv[rank == 1]
    d3[inv[rank == 2]] = dv[rank == 2]
    okey = -((d3 != 255).astype(np.int64) * 2 + (d2 != 255).astype(np.int64))
    so = np.argsort(okey, kind="stable")
    return (slot_src[so], d1[so], d2[so], d3[so],
            int((d2 != 255).sum()), int((d3 != 255).sum()))


def _pack_pairs(vals, ctot):
    cols = vals.reshape(ctot, P).T  # [p, c]
    return np.repeat(cols[:, :, None], 2, axis=2).astype(NPBF16).copy()


def _prep_edges(edge_index, N, keeps):
    """Sort kept edges by dst block, dedup (block, src) into <=3-dst slots.

    keeps: per-core boolean [N] — edges whose dst is unmasked in every batch
    of that core are dead and dropped. Slot counts are padded to common
    per-block maxima so all cores run one program; padding slots read the
    zero row (index N) with d1=0 (zero contribution), d2=d3=255 (sentinel).

    Returns (nch, m2ch, m3ch, idx_list, dl1_list, dl2_list, dl3_list).
    """
    NB = (N + P - 1) // P
    src = np.asarray(edge_index[0], np.int64)
    dst = np.asarray(edge_index[1], np.int64)
    order = np.argsort(dst, kind="stable")
    ss, ds = src[order], dst[order]
    per_core = []
    for keep in keeps:
        sel = keep[ds]
        ss_c, ds_c = ss[sel], ds[sel]
        bounds = np.searchsorted(ds_c, np.arange(0, (NB + 1) * P, P).clip(max=N))
        blocks = []
        for b in range(NB):
            lo, hi = bounds[b], bounds[b + 1]
            blocks.append(_slots(ss_c[lo:hi], ds_c[lo:hi] - b * P, N))
        per_core.append(blocks)
    nch, m2ch, m3ch = [], [], []
    for b in range(NB):
        nch.append(max(1, max(-(-len(pc[b][0]) // P) for pc in per_core)))
        m2ch.append(max(-(-pc[b][4] // P) for pc in per_core))
        m3ch.append(max(-(-pc[b][5] // P) for pc in per_core))
    ctot, ctot2, ctot3 = sum(nch), sum(m2ch), sum(m3ch)
    idx_list, dl1_list, dl2_list, dl3_list = [], [], [], []
    for blocks in per_core:
        srcs, dl1s, dl2s, dl3s = [], [], [], []
        for b in range(NB):
            sv, d1, d2, d3, _, _ = blocks[b]
            npad = nch[b] * P - len(sv)
            srcs.append(np.concatenate([sv, np.full(npad, N, np.int64)]))
            d1f = np.concatenate([d1, np.zeros(npad, np.int64)])
            d2f = np.concatenate([d2, np.full(npad, 255, np.int64)])
            d3f = np.concatenate([d3, np.full(npad, 255, np.int64)])
            dl1s.append(d1f)
            dl2s.append(d2f[:m2ch[b] * P])
            dl3s.append(d3f[:m3ch[b] * P])
        src_all = np.concatenate(srcs).astype(np.int16)
        iw = src_all.reshape(ctot * 8, 16).T  # index i -> [i%16, i//16]
        idx_list.append(np.tile(iw, (8, 1)).copy())
        dl1_list.append(_pack_pairs(
            np.concatenate(dl1s).astype(np.float32), ctot))
        dl2_list.append(_pack_pairs(
            np.concatenate(dl2s).astype(np.float32), max(ctot2, 1))
            if ctot2 else np.zeros((P, 1, 2), NPBF16))
        dl3_list.append(_pack_pairs(
            np.concatenate(dl3s).astype(np.float32), max(ctot3, 1))
            if ctot3 else np.zeros((P, 1, 2), NPBF16))
    return (tuple(nch), tuple(m2ch), tuple(m3ch),
            idx_list, dl1_list, dl2_list, dl3_list)


def _build(nch, m2ch, m3ch, B, N, H):
    """Build the Bass program (same for all cores)."""
    BPC = B // NCORES
    NB = (N + P - 1) // P
    NPAD = N + 16
    FD = BPC * H  # free-dim row size per node (256)
    ctot = sum(nch)
    ctot2 = max(1, sum(m2ch))
    ctot3 = max(1, sum(m3ch))
    coloff = np.concatenate([[0], np.cumsum(nch)]).astype(int)
    coloff2 = np.concatenate([[0], np.cumsum(m2ch)]).astype(int)
    coloff3 = np.concatenate([[0], np.cumsum(m3ch)]).astype(int)
    NBF = N // P  # full blocks
    rem = N - NBF * P

    nc = bacc.Bacc("TRN2", target_bir_lowering=False, debug=False, num_swdge_queues=2)

    x_in = nc.dram_tensor("x", [BPC, N, H], F32, kind="ExternalInput").ap()
    m_in = nc.dram_tensor("mask", [BPC, N], I32, kind="ExternalInput").ap()
    w1_in = nc.dram_tensor("W1", [2 * H, H], F32, kind="ExternalInput").ap()
    b1_in = nc.dram_tensor("b1", [H], F32, kind="ExternalInput").ap()
    w2_in = nc.dram_tensor("W2", [H, H], F32, kind="ExternalInput").ap()
    b2_in = nc.dram_tensor("b2", [H], F32, kind="ExternalInput").ap()
    idx_in = nc.dram_tensor("idx", [P, ctot * 8], I16, kind="ExternalInput").ap()
    dl_in = nc.dram_tensor("dl", [P, ctot, 2], BF16, kind="ExternalInput").ap()
    dl2_in = nc.dram_tensor("dl2", [P, ctot2, 2], BF16, kind="ExternalInput").ap()
    dl3_in = nc.dram_tensor("dl3", [P, ctot3, 2], BF16, kind="ExternalInput").ap()
    iota_in = nc.dram_tensor("iota", [P, P], BF16, kind="ExternalInput").ap()
    idb_in = nc.dram_tensor("idb", [P, P], BF16, kind="ExternalInput").ap()
    idf_in = nc.dram_tensor("idf", [P, P], F32, kind="ExternalInput").ap()
    y_out = nc.dram_tensor("y", [BPC, N, H], F32, kind="ExternalOutput").ap()

    with tile.TileContext(nc) as tc:
        with (
            tc.tile_pool(name="const", bufs=1) as cpool,
            tc.tile_pool(name="work", bufs=3) as wpool,
            tc.tile_pool(name="msgs", bufs=2) as mpool,
            tc.tile_pool(name="ohp", bufs=2) as opool,
            tc.tile_pool(name="idxp", bufs=2) as ipool,
            tc.tile_pool(name="psA", bufs=3, space="PSUM") as psA,
            tc.tile_pool(name="psC", bufs=1, space="PSUM") as psC,
            tc.tile_pool(name="psB", bufs=1, space="PSUM") as psB,
            tc.tile_pool(name="dram", bufs=1, space="DRAM") as dpool,
        ):
            # ---- constants / persistent tiles (deprioritized so the
            # gather table build wins the DMA engines during the
            # prologue; consumers on DVE/PE simply wait) ----
            with tc.high_priority(offset=-120):
                # ---- constants / persistent tiles ----
                dl_t = cpool.tile([P, ctot, 2], BF16, tag="dl")
                nc.sync.dma_start(out=dl_t[:], in_=dl_in)
                dl2_t = cpool.tile([P, ctot2, 2], BF16, tag="dl2")
                nc.sync.dma_start(out=dl2_t[:], in_=dl2_in)
                dl3_t = cpool.tile([P, ctot3, 2], BF16, tag="dl3")
                nc.sync.dma_start(out=dl3_t[:], in_=dl3_in)
                iota_t = cpool.tile([P, P], BF16, tag="iota")
                nc.sync.dma_start(out=iota_t[:], in_=iota_in)
                idb_t = cpool.tile([P, P], BF16, tag="idb")
                nc.sync.dma_start(out=idb_t[:], in_=idb_in)
                idf_t = cpool.tile([P, P], F32, tag="idf")
                nc.sync.dma_start(out=idf_t[:], in_=idf_in)

                w1a_f = cpool.tile([P, H], F32, tag="w1af")
                nc.sync.dma_start(out=w1a_f[:], in_=w1_in[0:H, :])
                w1b_f = cpool.tile([P, H], F32, tag="w1bf")
                nc.sync.dma_start(out=w1b_f[:], in_=w1_in[H:2 * H, :])
                w2_f = cpool.tile([P, H], F32, tag="w2f")
                nc.sync.dma_start(out=w2_f[:], in_=w2_in)

                b1c = cpool.tile([P, 1], F32, tag="b1c")
                nc.sync.dma_start(out=b1c[:], in_=b1_in[:, None])
                b2c = cpool.tile([P, 1], F32, tag="b2c")
                nc.sync.dma_start(out=b2c[:], in_=b2_in[:, None])

                # mask -> per-block per-partition columns, f32, and (1-m)
                mi = cpool.tile([P, NB, BPC], I32, tag="mi")
                for j in range(BPC):
                    nc.sync.dma_start(
                        out=mi[:, 0:NBF, j],
                        in_=m_in[j, 0:NBF * P].rearrange("(nb p) -> p nb", p=P),
                    )
                    if rem:
                        nc.sync.dma_start(
                            out=mi[:rem, NBF, j], in_=m_in[j, NBF * P:N][:, None]
                        )
                m_f = cpool.tile([P, NB, BPC], F32, tag="mf")
                nc.vector.tensor_copy(out=m_f[:], in_=mi[:])
                omm = cpool.tile([P, NB, BPC], F32, tag="omm")
                nc.vector.tensor_scalar(
                    out=omm[:], in0=m_f[:], scalar1=-1.0, scalar2=1.0,
                    op0=ALU.mult, op1=ALU.add,
                )

            # resident fp32 x, node-major per block: [p, nb, batch, h].
            # Loaded in row ranges; each range is cast to the bf16 gather
            # table as soon as its batches land, overlapping load and cast.
            x_res = cpool.tile([P, NB, BPC, H], F32, tag="xres")
            xbf = dpool.tile([NPAD, BPC, H], BF16, tag="xbf")
            RSPLIT = 4
            edges_r = [NBF * (i + 1) // RSPLIT for i in range(RSPLIT)]
            lo = 0
            for hi in edges_r:
                for j in range(BPC):
                    nc.sync.dma_start(
                        out=x_res[:, lo:hi, j, :],
                        in_=x_in[j, lo * P:hi * P, :].rearrange(
                            "(nb p) h -> p nb h", p=P
                        ),
                    )
                nc.gpsimd.dma_start(
                    out=xbf[lo * P:hi * P, :, :].rearrange(
                        "(nb p) b h -> p nb (b h)", p=P
                    ),
                    in_=x_res[:, lo:hi, :, :].rearrange("p nb b h -> p nb (b h)"),
                )
                lo = hi
            if rem:
                for j in range(BPC):
                    nc.sync.dma_start(
                        out=x_res[:rem, NBF, j, :],
                        in_=x_in[j, NBF * P:N, :],
                    )
                nc.gpsimd.dma_start(
                    out=xbf[NBF * P:N, :, :].rearrange("p b h -> p (b h)"),
                    in_=x_res[:rem, NBF, :, :].rearrange("p b h -> p (b h)"),
                )
            zrow = cpool.tile([16, FD], BF16, tag="zrow")
            nc.gpsimd.memset(zrow[:], 0.0)
            nc.sync.dma_start(
                out=xbf[N:NPAD, :, :].rearrange("r b h -> r (b h)"), in_=zrow[:]
            )

            # ---- main loop over dst blocks (gathers merged in pairs) ----
            xbf_flat = xbf[:].rearrange("r b h -> r (b h)")
            IDXG = 4  # blocks per idx-load DMA
            idx_tiles = {}
            pairs = [
                tuple(pb_ for pb_ in (b0, b0 + 1) if pb_ < NB)
                for b0 in range(0, NB, 2)
            ]
            for pair in pairs:
                b0 = pair[0]
                kp = sum(nch[b] for b in pair)
                poff = int(coloff[b0])

                if b0 % IDXG == 0:
                    ge = min(NB, b0 + IDXG)
                    goff = int(coloff[b0])
                    gk = int(coloff[ge]) - goff
                    gt = ipool.tile([P, gk * 8], I16, tag="idxs")
                    nc.sync.dma_start(
                        out=gt[:], in_=idx_in[:, goff * 8:(goff + gk) * 8]
                    )
                    idx_tiles = {"tile": gt, "goff": goff}
                loc = (poff - idx_tiles["goff"]) * 8
                ms = mpool.tile([P, kp, FD], BF16, tag="ms")
                nc.gpsimd.dma_gather(
                    out_ap=ms[:],
                    in_ap=xbf_flat,
                    idxs_ap=idx_tiles["tile"][:, loc:loc + kp * 8],
                    num_idxs=kp * P,
                    num_idxs_reg=kp * P,
                    elem_size=FD,
                    single_packet=False,
                    queue_num=(b0 // 2) % 2,
                )
                oh = opool.tile([P, kp, P], BF16, tag="oh")
                nc.vector.tensor_tensor(
                    out=oh[:].rearrange("p c (a t) -> p c a t", t=2),
                    in0=iota_t[:, None, :].to_broadcast([P, kp, P]).rearrange(
                        "p c (a t) -> p c a t", t=2
                    ),
                    in1=dl_t[:, poff:poff + kp, None, :].to_broadcast(
                        [P, kp, P // 2, 2]
                    ),
                    op=ALU.is_equal,
                )

                for b in pair:
                    lo = int(coloff[b]) - poff
                    for mch, dlx_t, cofx, tg in (
                        (m2ch[b], dl2_t, coloff2, "oh2"),
                        (m3ch[b], dl3_t, coloff3, "oh3"),
                    ):
                        if not mch:
                            continue
                        offx = int(cofx[b])
                        tx = opool.tile([P, mch, P], BF16, tag=tg)
                        nc.vector.tensor_tensor(
                            out=tx[:].rearrange("p c (a t) -> p c a t", t=2),
                            in0=iota_t[:, None, :].to_broadcast(
                                [P, mch, P]
                            ).rearrange("p c (a t) -> p c a t", t=2),
                            in1=dlx_t[:, offx:offx + mch, None, :].to_broadcast(
                                [P, mch, P // 2, 2]
                            ),
                            op=ALU.is_equal,
                        )
                        nc.vector.tensor_tensor(
                            out=oh[:, lo:lo + mch, :],
                            in0=oh[:, lo:lo + mch, :],
                            in1=tx[:],
                            op=ALU.add,
                        )

                for b in pair:
                    c0 = b * P
                    pb = min(P, N - c0)
                    k = nch[b]
                    lo = int(coloff[b]) - poff

                    nbp = psA.tile([P, BPC, H], F32, tag="nbp", space="PSUM")
                    for c in range(k):
                        nc.tensor.matmul(
                            out=nbp[:].rearrange("p b h -> p (b h)"),
                            lhsT=oh[:, lo + c, :],
                            rhs=ms[:, lo + c, :],
                            start=(c == 0),
                            stop=(c == k - 1),
                        )
                    nb_sb = wpool.tile([P, BPC, H], F32, tag="nb_sb")
                    nc.scalar.activation(out=nb_sb[:], in_=nbp[:], func=AF.Copy)

                    tpn = psB.tile([P, BPC, P], F32, tag="tpn", space="PSUM")
                    for j in range(BPC):
                        nc.tensor.transpose(
                            out=tpn[:, j, :], in_=nb_sb[:, j, :], identity=idf_t[:]
                        )
                    nT = wpool.tile([P, BPC, P], F32, tag="nT")
                    nc.scalar.activation(out=nT[:], in_=tpn[:], func=AF.Copy)

                    # per-block feature-major x from resident fp32 x
                    tpx = psB.tile([P, BPC, P], F32, tag="tpx", space="PSUM")
                    for j in range(BPC):
                        nc.tensor.transpose(
                            out=tpx[:, j, :pb], in_=x_res[:pb, b, j, :],
                            identity=idf_t[:pb, :pb],
                        )
                    xTb = wpool.tile([P, BPC, P], F32, tag="xTb")
                    nc.scalar.activation(
                        out=xTb[:, :, :pb], in_=tpx[:, :, :pb], func=AF.Copy
                    )

                    h_ps = psB.tile([P, BPC, H], F32, tag="h_ps", space="PSUM")
                    nc.tensor.matmul(
                        out=h_ps[:, :, :pb], lhsT=w1a_f[:], rhs=nT[:, :, :pb],
                        start=True, stop=False,
                    )
                    nc.tensor.matmul(
                        out=h_ps[:, :, :pb], lhsT=w1b_f[:], rhs=xTb[:, :, :pb],
                        start=False, stop=True,
                    )
                    h_sb = wpool.tile([P, BPC, H], F32, tag="h_sb")
                    nc.scalar.activation(
                        out=h_sb[:, :, :pb], in_=h_ps[:, :, :pb], func=AF.Relu,
                        bias=b1c[:],
                    )

                    imp_ps = psB.tile([P, BPC, H], F32, tag="imp_ps", space="PSUM")
                    nc.tensor.matmul(
                        out=imp_ps[:, :, :pb], lhsT=w2_f[:],
                        rhs=h_sb[:, :, :pb],
                        start=True, stop=True,
                    )
                    imp_sb = wpool.tile([P, BPC, H], F32, tag="imp_sb")
                    nc.vector.tensor_scalar(
                        out=imp_sb[:, :, :pb], in0=imp_ps[:, :, :pb],
                        scalar1=b2c[:], scalar2=None, op0=ALU.add,
                    )

                    outT = psC.tile([P, BPC, P], F32, tag="outT", space="PSUM")
                    for j in range(BPC):
                        nc.tensor.transpose(
                            out=outT[:pb, j, :], in_=imp_sb[:, j, :pb],
                            identity=idf_t[:],
                        )

                    xt_sb = wpool.tile([P, BPC, H], F32, tag="xt_sb")
                    for j in range(BPC):
                        nc.scalar.activation(
                            out=xt_sb[:pb, j, :], in_=x_res[:pb, b, j, :],
                            func=AF.Copy, scale=omm[:pb, b:b + 1, j],
                        )
                    out_sb = wpool.tile([P, BPC, H], F32, tag="out_sb")
                    for j in range(BPC):
                        nc.vector.scalar_tensor_tensor(
                            out=out_sb[:pb, j, :],
                            in0=outT[:pb, j, :],
                            scalar=m_f[:pb, b:b + 1, j],
                            in1=xt_sb[:pb, j, :],
                            op0=ALU.mult,
                            op1=ALU.add,
                        )
                    nc.sync.dma_start(
                        out=y_out[:, c0:c0 + pb, :].rearrange("b p h -> p b h"),
                        in_=out_sb[:pb],
                    )

    nc.compile()
    return nc


def kernel(node_embeddings, missing_mask, edge_index, W1, b1, W2, b2, trace=False):
    global last_results
    x = np.ascontiguousarray(np.asarray(node_embeddings, np.float32))
    mask = np.ascontiguousarray(np.asarray(missing_mask, np.int32))
    B, N, H = x.shape
    assert H == P and B % NCORES == 0
    BPC = B // NCORES

    ekey = (N, B, H, hash(np.asarray(edge_index).tobytes()),
            hash(mask.tobytes()))
    if ekey not in _cache:
        keeps = [
            (mask[c * BPC:(c + 1) * BPC] != 0).any(axis=0) for c in range(NCORES)
        ]
        nch, m2ch, m3ch, idx_list, dl1_list, dl2_list, dl3_list = _prep_edges(
            edge_index, N, keeps
        )
        nc = _build(nch, m2ch, m3ch, B, N, H)
        _cache[ekey] = (nc, idx_list, dl1_list, dl2_list, dl3_list)
    nc, idx_list, dl1_list, dl2_list, dl3_list = _cache[ekey]

    iota_arr = np.broadcast_to(
        np.arange(P, dtype=np.float32), (P, P)
    ).astype(NPBF16).copy()
    idb_arr = np.eye(P, dtype=np.float32).astype(NPBF16)
    idf_arr = np.eye(P, dtype=np.float32)

    common = {
        "W1": np.ascontiguousarray(np.asarray(W1, np.float32)),
        "b1": np.ascontiguousarray(np.asarray(b1, np.float32)),
        "W2": np.ascontiguousarray(np.asarray(W2, np.float32)),
        "b2": np.ascontiguousarray(np.asarray(b2, np.float32)),
        "iota": iota_arr,
        "idb": idb_arr,
        "idf": idf_arr,
    }
    in_maps = []
    for c in range(NCORES):
        m = dict(common)
        m["x"] = np.ascontiguousarray(x[c * BPC:(c + 1) * BPC])
        m["mask"] = np.ascontiguousarray(mask[c * BPC:(c + 1) * BPC])
        m["idx"] = idx_list[c]
        m["dl"] = dl1_list[c]
        m["dl2"] = dl2_list[c]
        m["dl3"] = dl3_list[c]
        in_maps.append(m)

    try:
        res = bass_utils.run_bass_kernel_spmd(
            nc, in_maps, core_ids=list(range(NCORES)), trace=trace
        )
    except ModuleNotFoundError:
        res = bass_utils.run_bass_kernel_spmd(
            nc, in_maps, core_ids=list(range(NCORES)), trace=False
        )
    last_results = res
    return np.concatenate([res.results[c]["y"] for c in range(NCORES)], axis=0)



# revision 9
# speedup vs baseline: 1.0537x; 1.0537x over previous
"""Trainium2 Bass kernel for MissingSensorImputation (GNN message passing).

Strategy (per core, data-parallel over batch: 8 cores x 2 batches):
  - One SWDGE casting DMA reorganizes x into a bf16 HBM table [node, 2*H]
    (512B rows) for gathering.
  - Edges sorted by dst, grouped per 128-node dst block, padded to chunks of
    128 edges. dma_gather pulls each chunk's src rows into SBUF partitions.
  - Scatter-add = PE matmul: onehot(dstlocal)^T @ msgs accumulated in PSUM
    per dst block (onehot generated on DVE via iota==dstlocal compare in
    2x_1p-friendly paired layout).
  - MLP (feature-major): transpose neighbor and x per block on PE,
    h=relu(W1a@nT + W1b@xT + b1), imputed = W2@h + b2, transpose back,
    blend with resident fp32 x by the missing mask.
Host-side preprocessing touches only the edge index (sorting/padding) and
constant tables; all tensor math runs on device.
"""
import sys

sys.path.insert(0, "/opt/trn_rl_repo")

import numpy as np
import concourse.bacc as bacc
import concourse.mybir as mybir
import concourse.tile as tile
from concourse import bass_utils

P = 128
NCORES = 8
F32 = mybir.dt.float32
BF16 = mybir.dt.bfloat16
I16 = mybir.dt.int16
I32 = mybir.dt.int32
NPBF16 = mybir.dt.np(BF16)

AF = mybir.ActivationFunctionType
ALU = mybir.AluOpType

_cache = {}
last_results = None  # BassKernelResults of the most recent run (for profiling)


def _slots(s_b, d_b, N):
    """Group a block's edges by src into slots carrying up to 3 dst columns.

    Returns (src, d1, d2, d3, n2, n3): slot source rows, up to three
    dstlocal values per slot (255 = unused sentinel), and the counts of
    slots having a 2nd / 3rd dst. Slots with more dsts sort first so d2/d3
    occupy a prefix.
    """
    o = np.argsort(s_b, kind="stable")
    sv, dv = s_b[o], d_b[o]
    n = len(sv)
    if n == 0:
        z = np.zeros(0, np.int64)
        return z, z, z, z, 0, 0
    new = np.r_[True, sv[1:] != sv[:-1]]
    runstart = np.maximum.accumulate(np.where(new, np.arange(n), 0))
    pos = np.arange(n) - runstart
    sub = pos // 3
    rank = pos % 3
    runid = np.cumsum(new) - 1
    skey = runid * (n + 1) + sub
    uniq, inv = np.unique(skey, return_inverse=True)
    ns = len(uniq)
    slot_src = np.zeros(ns, np.int64)
    d1 = np.zeros(ns, np.int64)
    d2 = np.full(ns, 255, np.int64)
    d3 = np.full(ns, 255, np.int64)
    slot_src[inv] = sv
    d1[inv[rank == 0]] = dv[rank == 0]
    d2[inv[rank == 1]] = dv[rank == 1]
    d3[inv[rank == 2]] = dv[rank == 2]
    okey = -((d3 != 255).astype(np.int64) * 2 + (d2 != 255).astype(np.int64))
    so = np.argsort(okey, kind="stable")
    return (slot_src[so], d1[so], d2[so], d3[so],
            int((d2 != 255).sum()), int((d3 != 255).sum()))


def _pack_pairs(vals, ctot):
    cols = vals.reshape(ctot, P).T  # [p, c]
    return np.repeat(cols[:, :, None], 2, axis=2).astype(NPBF16).copy()


def _prep_edges(edge_index, N, keeps):
    """Sort kept edges by dst block, dedup (block, src) into <=3-dst slots.

    keeps: per-core boolean [N] — edges whose dst is unmasked in every batch
    of that core are dead and dropped. Slot counts are padded to common
    per-block maxima so all cores run one program; padding slots read the
    zero row (index N) with d1=0 (zero contribution), d2=d3=255 (sentinel).

    Returns (nch, m2ch, m3ch, idx_list, dl1_list, dl2_list, dl3_list).
    """
    NB = (N + P - 1) // P
    src = np.asarray(edge_index[0], np.int64)
    dst = np.asarray(edge_index[1], np.int64)
    order = np.argsort(dst, kind="stable")
    ss, ds = src[order], dst[order]
    per_core = []
    for keep in keeps:
        sel = keep[ds]
        ss_c, ds_c = ss[sel], ds[sel]
        bounds = np.searchsorted(ds_c, np.arange(0, (NB + 1) * P, P).clip(max=N))
        blocks = []
        for b in range(NB):
            lo, hi = bounds[b], bounds[b + 1]
            blocks.append(_slots(ss_c[lo:hi], ds_c[lo:hi] - b * P, N))
        per_core.append(blocks)
    nch, m2ch, m3ch = [], [], []
    for b in range(NB):
        nch.append(max(1, max(-(-len(pc[b][0]) // P) for pc in per_core)))
        m2ch.append(max(-(-pc[b][4] // P) for pc in per_core))
        m3ch.append(max(-(-pc[b][5] // P) for pc in per_core))
    ctot, ctot2, ctot3 = sum(nch), sum(m2ch), sum(m3ch)
    idx_list, dl1_list, dl2_list, dl3_list = [], [], [], []
    for blocks in per_core:
        srcs, dl1s, dl2s, dl3s = [], [], [], []
        for b in range(NB):
            sv, d1, d2, d3, _, _ = blocks[b]
            npad = nch[b] * P - len(sv)
            srcs.append(np.concatenate([sv, np.full(npad, N, np.int64)]))
            d1f = np.concatenate([d1, np.zeros(npad, np.int64)])
            d2f = np.concatenate([d2, np.full(npad, 255, np.int64)])
            d3f = np.concatenate([d3, np.full(npad, 255, np.int64)])
            dl1s.append(d1f)
            dl2s.append(d2f[:m2ch[b] * P])
            dl3s.append(d3f[:m3ch[b] * P])
        src_all = np.concatenate(srcs).astype(np.int16)
        iw = src_all.reshape(ctot * 8, 16).T  # index i -> [i%16, i//16]
        idx_list.append(np.tile(iw, (8, 1)).copy())
        dl1_list.append(_pack_pairs(
            np.concatenate(dl1s).astype(np.float32), ctot))
        dl2_list.append(_pack_pairs(
            np.concatenate(dl2s).astype(np.float32), max(ctot2, 1))
            if ctot2 else np.zeros((P, 1, 2), NPBF16))
        dl3_list.append(_pack_pairs(
            np.concatenate(dl3s).astype(np.float32), max(ctot3, 1))
            if ctot3 else np.zeros((P, 1, 2), NPBF16))
    return (tuple(nch), tuple(m2ch), tuple(m3ch),
            idx_list, dl1_list, dl2_list, dl3_list)


def _build(nch, m2ch, m3ch, B, N, H):
    """Build the Bass program (same for all cores)."""
    BPC = B // NCORES
    NB = (N + P - 1) // P
    NPAD = N + 16
    FD = BPC * H  # free-dim row size per node (256)
    ctot = sum(nch)
    ctot2 = max(1, sum(m2ch))
    ctot3 = max(1, sum(m3ch))
    coloff = np.concatenate([[0], np.cumsum(nch)]).astype(int)
    coloff2 = np.concatenate([[0], np.cumsum(m2ch)]).astype(int)
    coloff3 = np.concatenate([[0], np.cumsum(m3ch)]).astype(int)
    NBF = N // P  # full blocks
    rem = N - NBF * P

    nc = bacc.Bacc("TRN2", target_bir_lowering=False, debug=False, num_swdge_queues=2)

    x_in = nc.dram_tensor("x", [BPC, N, H], F32, kind="ExternalInput").ap()
    m_in = nc.dram_tensor("mask", [BPC, N], I32, kind="ExternalInput").ap()
    w1a_in = nc.dram_tensor("W1a", [H, H], BF16, kind="ExternalInput").ap()
    w1b_in = nc.dram_tensor("W1b", [H, H], BF16, kind="ExternalInput").ap()
    w2_in = nc.dram_tensor("W2b", [H, H], BF16, kind="ExternalInput").ap()
    b1_in = nc.dram_tensor("b1", [H], F32, kind="ExternalInput").ap()
    b2_in = nc.dram_tensor("b2", [H], F32, kind="ExternalInput").ap()
    idx_in = nc.dram_tensor("idx", [P, ctot * 8], I16, kind="ExternalInput").ap()
    dl_in = nc.dram_tensor("dl", [P, ctot, 2], BF16, kind="ExternalInput").ap()
    dl2_in = nc.dram_tensor("dl2", [P, ctot2, 2], BF16, kind="ExternalInput").ap()
    dl3_in = nc.dram_tensor("dl3", [P, ctot3, 2], BF16, kind="ExternalInput").ap()
    iota_in = nc.dram_tensor("iota", [P, P], BF16, kind="ExternalInput").ap()
    idb_in = nc.dram_tensor("idb", [P, P], BF16, kind="ExternalInput").ap()
    idf_in = nc.dram_tensor("idf", [P, P], F32, kind="ExternalInput").ap()
    y_out = nc.dram_tensor("y", [BPC, N, H], F32, kind="ExternalOutput").ap()

    with tile.TileContext(nc) as tc:
        with (
            tc.tile_pool(name="const", bufs=1) as cpool,
            tc.tile_pool(name="work", bufs=3) as wpool,
            tc.tile_pool(name="msgs", bufs=2) as mpool,
            tc.tile_pool(name="ohp", bufs=2) as opool,
            tc.tile_pool(name="idxp", bufs=2) as ipool,
            tc.tile_pool(name="psA", bufs=2, space="PSUM") as psA,
            tc.tile_pool(name="psA2", bufs=2, space="PSUM") as psA2,
            tc.tile_pool(name="psC", bufs=1, space="PSUM") as psC,
            tc.tile_pool(name="psB", bufs=1, space="PSUM") as psB,
            tc.tile_pool(name="dram", bufs=1, space="DRAM") as dpool,
        ):
            # ---- constants / persistent tiles (deprioritized so the
            # gather table build wins the DMA engines during the
            # prologue; consumers on DVE/PE simply wait) ----
            with tc.high_priority(offset=-120):
                # ---- constants / persistent tiles ----
                dl_t = cpool.tile([P, ctot, 2], BF16, tag="dl")
                nc.sync.dma_start(out=dl_t[:], in_=dl_in)
                dl2_t = cpool.tile([P, ctot2, 2], BF16, tag="dl2")
                nc.sync.dma_start(out=dl2_t[:], in_=dl2_in)
                dl3_t = cpool.tile([P, ctot3, 2], BF16, tag="dl3")
                nc.sync.dma_start(out=dl3_t[:], in_=dl3_in)
                iota_t = cpool.tile([P, P], BF16, tag="iota")
                nc.sync.dma_start(out=iota_t[:], in_=iota_in)
                idb_t = cpool.tile([P, P], BF16, tag="idb")
                nc.sync.dma_start(out=idb_t[:], in_=idb_in)
                idf_t = cpool.tile([P, P], F32, tag="idf")
                nc.sync.dma_start(out=idf_t[:], in_=idf_in)

                w1a_t = cpool.tile([P, H], BF16, tag="w1a")
                nc.sync.dma_start(out=w1a_t[:], in_=w1a_in)
                w1b_t = cpool.tile([P, H], BF16, tag="w1b")
                nc.sync.dma_start(out=w1b_t[:], in_=w1b_in)
                w2_t = cpool.tile([P, H], BF16, tag="w2b")
                nc.sync.dma_start(out=w2_t[:], in_=w2_in)

                b1c = cpool.tile([P, 1], F32, tag="b1c")
                nc.sync.dma_start(out=b1c[:], in_=b1_in[:, None])
                b2c = cpool.tile([P, 1], F32, tag="b2c")
                nc.sync.dma_start(out=b2c[:], in_=b2_in[:, None])

                # mask -> per-block per-partition columns, f32, and (1-m)
                mi = cpool.tile([P, NB, BPC], I32, tag="mi")
                for j in range(BPC):
                    nc.sync.dma_start(
                        out=mi[:, 0:NBF, j],
                        in_=m_in[j, 0:NBF * P].rearrange("(nb p) -> p nb", p=P),
                    )
                    if rem:
                        nc.sync.dma_start(
                            out=mi[:rem, NBF, j], in_=m_in[j, NBF * P:N][:, None]
                        )
                m_f = cpool.tile([P, NB, BPC], F32, tag="mf")
                nc.vector.tensor_copy(out=m_f[:], in_=mi[:])
                omm = cpool.tile([P, NB, BPC], F32, tag="omm")
                nc.vector.tensor_scalar(
                    out=omm[:], in0=m_f[:], scalar1=-1.0, scalar2=1.0,
                    op0=ALU.mult, op1=ALU.add,
                )

            # resident fp32 x, node-major per block: [p, nb, batch, h].
            # Loaded in row ranges; each range is cast to the bf16 gather
            # table as soon as its batches land, overlapping load and cast.
            x_res = cpool.tile([P, NB, BPC, H], F32, tag="xres")
            xbf = dpool.tile([NPAD, BPC, H], BF16, tag="xbf")
            RSPLIT = 4
            edges_r = [NBF * (i + 1) // RSPLIT for i in range(RSPLIT)]
            lo = 0
            for hi in edges_r:
                for j in range(BPC):
                    nc.sync.dma_start(
                        out=x_res[:, lo:hi, j, :],
                        in_=x_in[j, lo * P:hi * P, :].rearrange(
                            "(nb p) h -> p nb h", p=P
                        ),
                    )
                nc.gpsimd.dma_start(
                    out=xbf[lo * P:hi * P, :, :].rearrange(
                        "(nb p) b h -> p nb (b h)", p=P
                    ),
                    in_=x_res[:, lo:hi, :, :].rearrange("p nb b h -> p nb (b h)"),
                )
                lo = hi
            if rem:
                for j in range(BPC):
                    nc.sync.dma_start(
                        out=x_res[:rem, NBF, j, :],
                        in_=x_in[j, NBF * P:N, :],
                    )
                nc.gpsimd.dma_start(
                    out=xbf[NBF * P:N, :, :].rearrange("p b h -> p (b h)"),
                    in_=x_res[:rem, NBF, :, :].rearrange("p b h -> p (b h)"),
                )
            zrow = cpool.tile([16, FD], BF16, tag="zrow")
            nc.gpsimd.memset(zrow[:], 0.0)
            nc.sync.dma_start(
                out=xbf[N:NPAD, :, :].rearrange("r b h -> r (b h)"), in_=zrow[:]
            )

            # ---- main loop over dst blocks (gathers merged in pairs) ----
            xbf_flat = xbf[:].rearrange("r b h -> r (b h)")
            IDXG = 4  # blocks per idx-load DMA
            idx_tiles = {}
            pairs = [
                tuple(pb_ for pb_ in (b0, b0 + 1) if pb_ < NB)
                for b0 in range(0, NB, 2)
            ]
            for pair in pairs:
                b0 = pair[0]
                kp = sum(nch[b] for b in pair)
                poff = int(coloff[b0])

                if b0 % IDXG == 0:
                    ge = min(NB, b0 + IDXG)
                    goff = int(coloff[b0])
                    gk = int(coloff[ge]) - goff
                    gt = ipool.tile([P, gk * 8], I16, tag="idxs")
                    nc.sync.dma_start(
                        out=gt[:], in_=idx_in[:, goff * 8:(goff + gk) * 8]
                    )
                    idx_tiles = {"tile": gt, "goff": goff}
                loc = (poff - idx_tiles["goff"]) * 8
                ms = mpool.tile([P, kp, FD], BF16, tag="ms")
                nc.gpsimd.dma_gather(
                    out_ap=ms[:],
                    in_ap=xbf_flat,
                    idxs_ap=idx_tiles["tile"][:, loc:loc + kp * 8],
                    num_idxs=kp * P,
                    num_idxs_reg=kp * P,
                    elem_size=FD,
                    single_packet=False,
                    queue_num=(b0 // 2) % 2,
                )
                oh = opool.tile([P, kp, P], BF16, tag="oh")
                nc.vector.tensor_tensor(
                    out=oh[:].rearrange("p c (a t) -> p c a t", t=2),
                    in0=iota_t[:, None, :].to_broadcast([P, kp, P]).rearrange(
                        "p c (a t) -> p c a t", t=2
                    ),
                    in1=dl_t[:, poff:poff + kp, None, :].to_broadcast(
                        [P, kp, P // 2, 2]
                    ),
                    op=ALU.is_equal,
                )

                for b in pair:
                    lo = int(coloff[b]) - poff
                    for mch, dlx_t, cofx, tg in (
                        (m2ch[b], dl2_t, coloff2, "oh2"),
                        (m3ch[b], dl3_t, coloff3, "oh3"),
                    ):
                        if not mch:
                            continue
                        offx = int(cofx[b])
                        tx = opool.tile([P, mch, P], BF16, tag=tg)
                        nc.vector.tensor_tensor(
                            out=tx[:].rearrange("p c (a t) -> p c a t", t=2),
                            in0=iota_t[:, None, :].to_broadcast(
                                [P, mch, P]
                            ).rearrange("p c (a t) -> p c a t", t=2),
                            in1=dlx_t[:, offx:offx + mch, None, :].to_broadcast(
                                [P, mch, P // 2, 2]
                            ),
                            op=ALU.is_equal,
                        )
                        nc.vector.tensor_tensor(
                            out=oh[:, lo:lo + mch, :],
                            in0=oh[:, lo:lo + mch, :],
                            in1=tx[:],
                            op=ALU.add,
                        )

                for b in pair:
                    c0 = b * P
                    pb = min(P, N - c0)
                    k = nch[b]
                    lo = int(coloff[b]) - poff

                    # transposed scatter: nT[feat, dst] = sum_slot
                    # msgs[slot, feat] * onehot[slot, dst], per batch.
                    # One accumulation chain per PSUM bank: interleaved
                    # chains sharing a bank lose the first chunk on HW.
                    nTp0 = psA.tile([P, 512], F32, tag="nTp0", space="PSUM")
                    nTp1 = psA2.tile([P, 512], F32, tag="nTp1", space="PSUM")
                    nTp = [nTp0, nTp1]
                    for c in range(k):
                        for j in range(BPC):
                            nc.tensor.matmul(
                                out=nTp[j][:, 0:P],
                                lhsT=ms[:, lo + c, j * H:(j + 1) * H],
                                rhs=oh[:, lo + c, :],
                                start=(c == 0),
                                stop=(c == k - 1),
                            )
                    nT = wpool.tile([P, BPC, P], BF16, tag="nT")
                    for j in range(BPC):
                        nc.scalar.activation(
                            out=nT[:, j, :], in_=nTp[j][:, 0:P], func=AF.Copy
                        )

                    # per-block feature-major x from resident fp32 x
                    tpx = psB.tile([P, BPC, P], F32, tag="tpx", space="PSUM")
                    for j in range(BPC):
                        nc.tensor.transpose(
                            out=tpx[:, j, :pb], in_=x_res[:pb, b, j, :],
                            identity=idf_t[:pb, :pb],
                        )
                    xTb = wpool.tile([P, BPC, P], BF16, tag="xTb")
                    nc.scalar.activation(
                        out=xTb[:, :, :pb], in_=tpx[:, :, :pb], func=AF.Copy
                    )

                    h_ps = psB.tile([P, BPC, H], F32, tag="h_ps", space="PSUM")
                    for j in range(BPC):
                        nc.tensor.matmul(
                            out=h_ps[:, j, :pb], lhsT=w1a_t[:], rhs=nT[:, j, :pb],
                            start=True, stop=False,
                        )
                        nc.tensor.matmul(
                            out=h_ps[:, j, :pb], lhsT=w1b_t[:], rhs=xTb[:, j, :pb],
                            start=False, stop=True,
                        )
                    h_sb = wpool.tile([P, BPC, H], BF16, tag="h_sb")
                    nc.scalar.activation(
                        out=h_sb[:, :, :pb], in_=h_ps[:, :, :pb], func=AF.Relu,
                        bias=b1c[:],
                    )

                    imp_ps = psB.tile([P, BPC, H], F32, tag="imp_ps", space="PSUM")
                    for j in range(BPC):
                        nc.tensor.matmul(
                            out=imp_ps[:, j, :pb], lhsT=w2_t[:],
                            rhs=h_sb[:, j, :pb],
                            start=True, stop=True,
                        )
                    imp_sb = wpool.tile([P, BPC, H], BF16, tag="imp_sb")
                    nc.vector.tensor_scalar(
                        out=imp_sb[:, :, :pb], in0=imp_ps[:, :, :pb],
                        scalar1=b2c[:], scalar2=None, op0=ALU.add,
                    )

                    outT = psC.tile([P, BPC, P], BF16, tag="outT", space="PSUM")
                    for j in range(BPC):
                        nc.tensor.transpose(
                            out=outT[:pb, j, :], in_=imp_sb[:, j, :pb],
                            identity=idb_t[:],
                        )

                    xt_sb = wpool.tile([P, BPC, H], F32, tag="xt_sb")
                    for j in range(BPC):
                        nc.scalar.activation(
                            out=xt_sb[:pb, j, :], in_=x_res[:pb, b, j, :],
                            func=AF.Copy, scale=omm[:pb, b:b + 1, j],
                        )
                    out_sb = wpool.tile([P, BPC, H], F32, tag="out_sb")
                    for j in range(BPC):
                        nc.vector.scalar_tensor_tensor(
                            out=out_sb[:pb, j, :],
                            in0=outT[:pb, j, :],
                            scalar=m_f[:pb, b:b + 1, j],
                            in1=xt_sb[:pb, j, :],
                            op0=ALU.mult,
                            op1=ALU.add,
                        )
                    nc.sync.dma_start(
                        out=y_out[:, c0:c0 + pb, :].rearrange("b p h -> p b h"),
                        in_=out_sb[:pb],
                    )

    nc.compile()
    return nc


def kernel(node_embeddings, missing_mask, edge_index, W1, b1, W2, b2, trace=False):
    global last_results
    x = np.ascontiguousarray(np.asarray(node_embeddings, np.float32))
    mask = np.ascontiguousarray(np.asarray(missing_mask, np.int32))
    B, N, H = x.shape
    assert H == P and B % NCORES == 0
    BPC = B // NCORES

    ekey = (N, B, H, hash(np.asarray(edge_index).tobytes()),
            hash(mask.tobytes()))
    if ekey not in _cache:
        keeps = [
            (mask[c * BPC:(c + 1) * BPC] != 0).any(axis=0) for c in range(NCORES)
        ]
        nch, m2ch, m3ch, idx_list, dl1_list, dl2_list, dl3_list = _prep_edges(
            edge_index, N, keeps
        )
        nc = _build(nch, m2ch, m3ch, B, N, H)
        _cache[ekey] = (nc, idx_list, dl1_list, dl2_list, dl3_list)
    nc, idx_list, dl1_list, dl2_list, dl3_list = _cache[ekey]

    iota_arr = np.broadcast_to(
        np.arange(P, dtype=np.float32), (P, P)
    ).astype(NPBF16).copy()
    idb_arr = np.eye(P, dtype=np.float32).astype(NPBF16)
    idf_arr = np.eye(P, dtype=np.float32)

    W1f = np.asarray(W1, np.float32)
    W2f = np.asarray(W2, np.float32)
    common = {
        "W1a": np.ascontiguousarray(W1f[0:H].astype(NPBF16)),
        "W1b": np.ascontiguousarray(W1f[H:2 * H].astype(NPBF16)),
        "W2b": np.ascontiguousarray(W2f.astype(NPBF16)),
        "b1": np.ascontiguousarray(np.asarray(b1, np.float32)),
        "b2": np.ascontiguousarray(np.asarray(b2, np.float32)),
        "iota": iota_arr,
        "idb": idb_arr,
        "idf": idf_arr,
    }
    in_maps = []
    for c in range(NCORES):
        m = dict(common)
        m["x"] = np.ascontiguousarray(x[c * BPC:(c + 1) * BPC])
        m["mask"] = np.ascontiguousarray(mask[c * BPC:(c + 1) * BPC])
        m["idx"] = idx_list[c]
        m["dl"] = dl1_list[c]
        m["dl2"] = dl2_list[c]
        m["dl3"] = dl3_list[c]
        in_maps.append(m)

    try:
        res = bass_utils.run_bass_kernel_spmd(
            nc, in_maps, core_ids=list(range(NCORES)), trace=trace
        )
    except ModuleNotFoundError:
        res = bass_utils.run_bass_kernel_spmd(
            nc, in_maps, core_ids=list(range(NCORES)), trace=False
        )
    last_results = res
    return np.concatenate([res.results[c]["y"] for c in range(NCORES)], axis=0)



# revision 13
# speedup vs baseline: 1.1640x; 1.1046x over previous
"""Trainium2 Bass kernel for MissingSensorImputation (GNN message passing).

Strategy (per core, data parallel over batch: 8 cores x 2 batches):
  - One SWDGE casting DMA reorganizes x into a bf16 HBM table [node, 2*H]
    (512B rows) for gathering.
  - Edges deduped per (src, 128-block PAIR): a src feeding both blocks of
    a 256-dst superblock is gathered once. Slots carry up to 3 dsts per
    block; grouped [A={b0}, C={b0,b1}, B={b1}] so each block's chunks are
    one contiguous range. dma_gather pulls chunk src rows into SBUF.
  - Scatter-add = PE matmul in transposed orientation: for each chunk and
    batch, lhsT = msgs[slot, feat], rhs = onehot[slot, dst] accumulating
    neighbor^T[feat, dst] in PSUM. Each accumulation chain owns a full
    PSUM bank (interleaved chains sharing a bank corrupt on HW).
  - Onehot built on DVE via iota==dlist compares (2x_1p paired layout);
    2nd/3rd dsts handled by add-passes over a d2/d3-first column prefix.
  - MLP fully bf16 feature-major: h = relu(W1a@nT + W1b@xT + b1),
    imputed = W2@h + b2, transpose back, blend with resident fp32 x by
    the missing mask.
Host-side preprocessing touches only the edge index (sorting/dedup) and
constant tables; all tensor math runs on device.
"""
import sys

sys.path.insert(0, "/opt/trn_rl_repo")

import numpy as np
import concourse.bacc as bacc
import concourse.mybir as mybir
import concourse.tile as tile
from concourse import bass_utils

P = 128
NCORES = 8
F32 = mybir.dt.float32
BF16 = mybir.dt.bfloat16
I16 = mybir.dt.int16
I32 = mybir.dt.int32
NPBF16 = mybir.dt.np(BF16)

AF = mybir.ActivationFunctionType
ALU = mybir.AluOpType

_cache = {}
last_results = None  # BassKernelResults of the most recent run (for profiling)


def _expand_slots(sv, dv, pair_lo, nblk):
    """Vectorized sub-slot expansion for one (core, pair): dedup (src, pair)
    into slots of up to 3 dsts per 128-block. Returns (src, dvals[s, 2, 3]
    local-dst-or-255, groupmask, weight)."""
    n = len(sv)
    if n == 0:
        z = np.zeros((0,), np.int64)
        return z, np.full((0, 2, 3), 255, np.int16), z, z
    dloc = dv - pair_lo
    blk = (dloc >= P).astype(np.int64)
    dhalf = dloc - blk * P
    o = np.lexsort((dhalf, blk, sv))
    s, b, d = sv[o], blk[o], dhalf[o]
    new_s = np.r_[True, s[1:] != s[:-1]]
    run_id = np.cumsum(new_s) - 1
    nruns = run_id[-1] + 1
    c = np.zeros((nruns, 2), np.int64)
    np.add.at(c, (run_id, b), 1)
    c0, c1 = c[:, 0], c[:, 1]
    nsub = np.maximum((c0 + 2) // 3, np.maximum((c1 + 2) // 3, 1))
    seg_new = np.r_[True, (s[1:] != s[:-1]) | (b[1:] != b[:-1])]
    seg_start = np.maximum.accumulate(np.where(seg_new, np.arange(n), 0))
    pos = np.arange(n) - seg_start
    slot_base = np.r_[0, np.cumsum(nsub)[:-1]]
    erow = slot_base[run_id] + pos // 3
    ecol = pos % 3
    tot = int(nsub.sum())
    dvals = np.full((tot, 2, 3), 255, np.int16)
    dvals[erow, b, ecol] = d.astype(np.int16)
    run_src = s[new_s]
    src = np.repeat(run_src, nsub)
    j = np.arange(tot) - np.repeat(slot_base, nsub)
    cnt0 = np.minimum(np.repeat(c0, nsub) - 3 * j, 3).clip(min=0)
    cnt1 = np.minimum(np.repeat(c1, nsub) - 3 * j, 3).clip(min=0)
    if nblk == 1:
        cnt1[:] = 0
    mask = (cnt0 > 0).astype(np.int64) + 2 * (cnt1 > 0).astype(np.int64)
    weight = np.maximum(cnt0, cnt1)
    keep = mask > 0
    return src[keep], dvals[keep], mask[keep], weight[keep]


def prep_pairs(edge_index, N, keeps):
    """Slot tables for the pair-deduped gather. Per-(pair,group) slot counts
    padded to the max across cores so one program serves all cores; columns
    per (pair, block) sorted d2/d3-first so dl2/dl3 cover a prefix."""
    NB = (N + P - 1) // P
    NCORES_ = len(keeps)
    pair_blocks = [
        tuple(b for b in (b0, b0 + 1) if b < NB) for b0 in range(0, NB, 2)
    ]
    src = np.asarray(edge_index[0], np.int64)
    dst = np.asarray(edge_index[1], np.int64)
    oall = np.argsort(dst, kind="stable")
    ss_all, ds_all = src[oall], dst[oall]

    GORDER = [1, 3, 2]  # A={b0}, C={both}, B={b1}
    data = []
    for keep in keeps:
        sel = keep[ds_all]
        ss_c, ds_c = ss_all[sel], ds_all[sel]
        bounds = np.searchsorted(
            ds_c, np.arange(0, (len(pair_blocks) + 1) * 2 * P, 2 * P
                            ).clip(max=N))
        per_pair = []
        for pi, blks in enumerate(pair_blocks):
            lo, hi = bounds[pi], bounds[pi + 1]
            s, dv, m, w = _expand_slots(
                ss_c[lo:hi], ds_c[lo:hi], 2 * P * pi, len(blks))
            groups = {}
            for g in GORDER[:3 if len(blks) == 2 else 1]:
                gs = np.nonzero(m == g)[0]
                o = np.argsort(-w[gs], kind="stable")
                groups[g] = (s[gs[o]], dv[gs[o]])
            per_pair.append(groups)
        data.append(per_pair)

    spec = dict(pairs=[], pair_blocks=pair_blocks)
    core_src = [[] for _ in range(NCORES_)]
    core_dv = [[] for _ in range(NCORES_)]
    col_off = 0
    dl2_off = 0
    dl3_off = 0
    ctot = 0
    col_take1 = []  # (chunk_global, blk_in_pair) per dl1 column
    col_take2 = []
    col_take3 = []
    for pi, blks in enumerate(pair_blocks):
        glist = GORDER[:3 if len(blks) == 2 else 1]
        nmax = {g: max(len(data[c][pi][g][0]) for c in range(NCORES_))
                for g in glist}
        tot = sum(nmax.values())
        kp = max(1, -(-tot // P))
        pair_src = np.full((NCORES_, kp * P), N, np.int64)
        pair_dv = np.full((NCORES_, kp * P, 2, 3), 255, np.int16)
        off = 0
        for g in glist:
            for cidx in range(NCORES_):
                s, dv = data[cidx][pi][g]
                pair_src[cidx, off:off + len(s)] = s
                pair_dv[cidx, off:off + len(s)] = dv
            off += nmax[g]
        nA = nmax.get(1, 0)
        nC = nmax.get(3, 0)
        nB = nmax.get(2, 0)
        rng = {}
        if len(blks) == 2:
            rng[0] = (0, -(-max(1, nA + nC) // P))
            rng[1] = (nA // P, -(-max(1, nA + nC + nB) // P))
        else:
            rng[0] = (0, kp)
        d2 = (pair_dv[:, :, :, 1] != 255)
        d3 = (pair_dv[:, :, :, 2] != 255)
        pblocks = []
        for bi, bg in enumerate(blks):
            clo, chi = rng[bi]
            cols = list(range(clo, chi))
            h2 = [bool(d2[:, c * P:(c + 1) * P, bi].any()) for c in cols]
            h3 = [bool(d3[:, c * P:(c + 1) * P, bi].any()) for c in cols]
            order = sorted(range(len(cols)),
                           key=lambda i: (not h2[i], not h3[i]))
            I2 = sum(h2)
            I3 = sum(h3)
            chunk_cols = [(cols[i], r) for r, i in enumerate(order)]
            chunk_cols.sort()
            pblocks.append(dict(
                blk_global=bg, col_off=col_off, ncols=len(cols), I2=I2,
                I3=I3, dl2_off=dl2_off, dl3_off=dl3_off,
                chunk_cols=chunk_cols))
            for i in order:
                col_take1.append((ctot + cols[i], bi))
            for i in order[:I2]:
                col_take2.append((ctot + cols[i], bi))
            for i in order[:I3]:
                col_take3.append((ctot + cols[i], bi))
            col_off += len(cols)
            dl2_off += I2
            dl3_off += I3
        for c in range(NCORES_):
            core_src[c].append(pair_src[c])
            core_dv[c].append(pair_dv[c])
        spec["pairs"].append(dict(kp=kp, blocks=pblocks, ctot0=ctot))
        ctot += kp
    spec["ctot"] = ctot
    spec["ncols"] = col_off
    spec["n2"] = dl2_off
    spec["n3"] = dl3_off

    idx_list, dl1_list, dl2_list, dl3_list = [], [], [], []
    t1 = np.array(col_take1, np.int64).reshape(-1, 2)
    t2 = np.array(col_take2, np.int64).reshape(-1, 2)
    t3 = np.array(col_take3, np.int64).reshape(-1, 2)
    for c in range(NCORES_):
        src_all = np.concatenate(core_src[c]).astype(np.int16)
        iw = src_all.reshape(ctot * 8, 16).T
        idx_list.append(np.tile(iw, (8, 1)).copy())
        dvc = np.concatenate(core_dv[c]).reshape(ctot, P, 2, 3)

        def pack(takes, k):
            if len(takes) == 0:
                return np.full((P, 1, 2), 255, NPBF16)
            cols = dvc[takes[:, 0], :, takes[:, 1], k].T  # [P, ncol]
            return np.repeat(
                cols[:, :, None], 2, axis=2).astype(np.float32).astype(
                NPBF16).copy()
        dl1_list.append(pack(t1, 0))
        dl2_list.append(pack(t2, 1))
        dl3_list.append(pack(t3, 2))
    return spec, idx_list, dl1_list, dl2_list, dl3_list


def _build(spec, B, N, H):
    """Build the Bass program (same for all cores)."""
    BPC = B // NCORES
    NB = (N + P - 1) // P
    NPAD = N + 16
    FD = BPC * H  # free-dim row size per node (256)
    ctot = spec["ctot"]
    ncols = spec["ncols"]
    n2 = max(1, spec["n2"])
    n3 = max(1, spec["n3"])
    NBF = N // P  # full blocks
    rem = N - NBF * P

    nc = bacc.Bacc("TRN2", target_bir_lowering=False, debug=False,
                   num_swdge_queues=2)

    x_in = nc.dram_tensor("x", [BPC, N, H], F32, kind="ExternalInput").ap()
    m_in = nc.dram_tensor("mask", [BPC, N], I32, kind="ExternalInput").ap()
    w1a_in = nc.dram_tensor("W1a", [H, H], BF16, kind="ExternalInput").ap()
    w1b_in = nc.dram_tensor("W1b", [H, H], BF16, kind="ExternalInput").ap()
    w2_in = nc.dram_tensor("W2b", [H, H], BF16, kind="ExternalInput").ap()
    b1_in = nc.dram_tensor("b1", [H], F32, kind="ExternalInput").ap()
    b2_in = nc.dram_tensor("b2", [H], F32, kind="ExternalInput").ap()
    idx_in = nc.dram_tensor("idx", [P, ctot * 8], I16, kind="ExternalInput").ap()
    dl_in = nc.dram_tensor("dl", [P, ncols, 2], BF16, kind="ExternalInput").ap()
    dl2_in = nc.dram_tensor("dl2", [P, n2, 2], BF16, kind="ExternalInput").ap()
    dl3_in = nc.dram_tensor("dl3", [P, n3, 2], BF16, kind="ExternalInput").ap()
    iota_in = nc.dram_tensor("iota", [P, P], BF16, kind="ExternalInput").ap()
    idb_in = nc.dram_tensor("idb", [P, P], BF16, kind="ExternalInput").ap()
    idf_in = nc.dram_tensor("idf", [P, P], F32, kind="ExternalInput").ap()
    y_out = nc.dram_tensor("y", [BPC, N, H], F32, kind="ExternalOutput").ap()

    with tile.TileContext(nc) as tc:
        with (
            tc.tile_pool(name="const", bufs=1) as cpool,
            tc.tile_pool(name="work", bufs=3) as wpool,
            tc.tile_pool(name="msgs", bufs=2) as mpool,
            tc.tile_pool(name="ohp", bufs=2) as opool,
            tc.tile_pool(name="idxp", bufs=2) as ipool,
            tc.tile_pool(name="psA", bufs=1, space="PSUM") as psA,
            tc.tile_pool(name="psA2", bufs=1, space="PSUM") as psA2,
            tc.tile_pool(name="psA3", bufs=1, space="PSUM") as psA3,
            tc.tile_pool(name="psA4", bufs=1, space="PSUM") as psA4,
            tc.tile_pool(name="psC", bufs=1, space="PSUM") as psC,
            tc.tile_pool(name="psB", bufs=1, space="PSUM") as psB,
            tc.tile_pool(name="dram", bufs=1, space="DRAM") as dpool,
        ):
            # ---- constants / persistent tiles (deprioritized so the
            # gather table build wins the DMA engines during the
            # prologue; consumers on DVE/PE simply wait) ----
            with tc.high_priority(offset=-120):
                dl_t = cpool.tile([P, ncols, 2], BF16, tag="dl")
                nc.sync.dma_start(out=dl_t[:], in_=dl_in)
                dl2_t = cpool.tile([P, n2, 2], BF16, tag="dl2")
                nc.sync.dma_start(out=dl2_t[:], in_=dl2_in)
                dl3_t = cpool.tile([P, n3, 2], BF16, tag="dl3")
                nc.sync.dma_start(out=dl3_t[:], in_=dl3_in)
                iota_t = cpool.tile([P, P], BF16, tag="iota")
                nc.sync.dma_start(out=iota_t[:], in_=iota_in)
                idb_t = cpool.tile([P, P], BF16, tag="idb")
                nc.sync.dma_start(out=idb_t[:], in_=idb_in)
                idf_t = cpool.tile([P, P], F32, tag="idf")
                nc.sync.dma_start(out=idf_t[:], in_=idf_in)

                w1a_t = cpool.tile([P, H], BF16, tag="w1a")
                nc.sync.dma_start(out=w1a_t[:], in_=w1a_in)
                w1b_t = cpool.tile([P, H], BF16, tag="w1b")
                nc.sync.dma_start(out=w1b_t[:], in_=w1b_in)
                w2_t = cpool.tile([P, H], BF16, tag="w2b")
                nc.sync.dma_start(out=w2_t[:], in_=w2_in)

                b1c = cpool.tile([P, 1], F32, tag="b1c")
                nc.sync.dma_start(out=b1c[:], in_=b1_in[:, None])
                b2c = cpool.tile([P, 1], F32, tag="b2c")
                nc.sync.dma_start(out=b2c[:], in_=b2_in[:, None])

                # mask -> per-block per-partition columns, f32, and (1-m)
                mi = cpool.tile([P, NB, BPC], I32, tag="mi")
                for j in range(BPC):
                    nc.sync.dma_start(
                        out=mi[:, 0:NBF, j],
                        in_=m_in[j, 0:NBF * P].rearrange("(nb p) -> p nb", p=P),
                    )
                    if rem:
                        nc.sync.dma_start(
                            out=mi[:rem, NBF, j], in_=m_in[j, NBF * P:N][:, None]
                        )
                m_f = cpool.tile([P, NB, BPC], F32, tag="mf")
                nc.vector.tensor_copy(out=m_f[:], in_=mi[:])
                omm = cpool.tile([P, NB, BPC], F32, tag="omm")
                nc.vector.tensor_scalar(
                    out=omm[:], in0=m_f[:], scalar1=-1.0, scalar2=1.0,
                    op0=ALU.mult, op1=ALU.add,
                )

            # resident fp32 x, node-major per block: [p, nb, batch, h].
            # Loaded in row ranges; each range is cast to the bf16 gather
            # table as soon as its batches land, overlapping load and cast.
            x_res = cpool.tile([P, NB, BPC, H], F32, tag="xres")
            xbf = dpool.tile([NPAD, BPC, H], BF16, tag="xbf")
            RSPLIT = 4
            edges_r = [NBF * (i + 1) // RSPLIT for i in range(RSPLIT)]
            lo = 0
            for hi in edges_r:
                for j in range(BPC):
                    nc.sync.dma_start(
                        out=x_res[:, lo:hi, j, :],
                        in_=x_in[j, lo * P:hi * P, :].rearrange(
                            "(nb p) h -> p nb h", p=P
                        ),
                    )
                nc.gpsimd.dma_start(
                    out=xbf[lo * P:hi * P, :, :].rearrange(
                        "(nb p) b h -> p nb (b h)", p=P
                    ),
                    in_=x_res[:, lo:hi, :, :].rearrange("p nb b h -> p nb (b h)"),
                )
                lo = hi
            if rem:
                for j in range(BPC):
                    nc.sync.dma_start(
                        out=x_res[:rem, NBF, j, :],
                        in_=x_in[j, NBF * P:N, :],
                    )
                nc.gpsimd.dma_start(
                    out=xbf[NBF * P:N, :, :].rearrange("p b h -> p (b h)"),
                    in_=x_res[:rem, NBF, :, :].rearrange("p b h -> p (b h)"),
                )
            zrow = cpool.tile([16, FD], BF16, tag="zrow")
            nc.gpsimd.memset(zrow[:], 0.0)
            nc.sync.dma_start(
                out=xbf[N:NPAD, :, :].rearrange("r b h -> r (b h)"), in_=zrow[:]
            )

            # ---- main loop over pairs ----
            xbf_flat = xbf[:].rearrange("r b h -> r (b h)")
            IDXG = 2  # pairs per idx-load DMA
            idx_tiles = {}
            chain_pools = [[psA, psA2], [psA3, psA4]]
            for pi, pair in enumerate(spec["pairs"]):
                kp = pair["kp"]
                ct0 = pair["ctot0"]

                if pi % IDXG == 0:
                    pe = min(len(spec["pairs"]), pi + IDXG)
                    goff = ct0
                    gk = (spec["pairs"][pe - 1]["ctot0"]
                          + spec["pairs"][pe - 1]["kp"]) - goff
                    gt = ipool.tile([P, gk * 8], I16, tag="idxs")
                    nc.sync.dma_start(
                        out=gt[:], in_=idx_in[:, goff * 8:(goff + gk) * 8]
                    )
                    idx_tiles = {"tile": gt, "goff": goff}
                loc = (ct0 - idx_tiles["goff"]) * 8
                ms = mpool.tile([P, kp, FD], BF16, tag="ms")
                nc.gpsimd.dma_gather(
                    out_ap=ms[:],
                    in_ap=xbf_flat,
                    idxs_ap=idx_tiles["tile"][:, loc:loc + kp * 8],
                    num_idxs=kp * P,
                    num_idxs_reg=kp * P,
                    elem_size=FD,
                    single_packet=False,
                    queue_num=pi % 2,
                )

                # onehot tiles per block of the pair
                ohs = []
                for bi, blk in enumerate(pair["blocks"]):
                    nb_c = blk["ncols"]
                    oh = opool.tile([P, nb_c, P], BF16, tag=f"oh{bi}",
                                    name=f"oh{bi}")
                    nc.vector.tensor_tensor(
                        out=oh[:].rearrange("p c (a t) -> p c a t", t=2),
                        in0=iota_t[:, None, :].to_broadcast(
                            [P, nb_c, P]).rearrange(
                            "p c (a t) -> p c a t", t=2),
                        in1=dl_t[:, blk["col_off"]:blk["col_off"] + nb_c,
                                 None, :].to_broadcast([P, nb_c, P // 2, 2]),
                        op=ALU.is_equal,
                    )
                    for I_x, off_x, dlx_t, tg in (
                        (blk["I2"], blk["dl2_off"], dl2_t, "oh2"),
                        (blk["I3"], blk["dl3_off"], dl3_t, "oh3"),
                    ):
                        if not I_x:
                            continue
                        tx = opool.tile([P, I_x, P], BF16, tag=tg, name=tg)
                        nc.vector.tensor_tensor(
                            out=tx[:].rearrange("p c (a t) -> p c a t", t=2),
                            in0=iota_t[:, None, :].to_broadcast(
                                [P, I_x, P]).rearrange(
                                "p c (a t) -> p c a t", t=2),
                            in1=dlx_t[:, off_x:off_x + I_x, None,
                                      :].to_broadcast([P, I_x, P // 2, 2]),
                            op=ALU.is_equal,
                        )
                        nc.vector.tensor_tensor(
                            out=oh[:, 0:I_x, :],
                            in0=oh[:, 0:I_x, :],
                            in1=tx[:],
                            op=ALU.add,
                        )
                    ohs.append(oh)

                for bi, blk in enumerate(pair["blocks"]):
                    b = blk["blk_global"]
                    c0 = b * P
                    pb = min(P, N - c0)
                    oh = ohs[bi]
                    ccols = blk["chunk_cols"]

                    # transposed scatter: one PSUM bank per (block, batch)
                    # accumulation chain.
                    nTp0 = chain_pools[bi][0].tile(
                        [P, 512], F32, tag=f"nTp{bi}0", space="PSUM",
                        name=f"nTp{bi}0")
                    nTp1 = chain_pools[bi][1].tile(
                        [P, 512], F32, tag=f"nTp{bi}1", space="PSUM",
                        name=f"nTp{bi}1")
                    nTp = [nTp0, nTp1]
                    nk = len(ccols)
                    assert nk > 0, f"empty chain for block {b}"
                    for ci, (c_local, col_rel) in enumerate(ccols):
                        for j in range(BPC):
                            nc.tensor.matmul(
                                out=nTp[j][:, 0:P],
                                lhsT=ms[:, c_local, j * H:(j + 1) * H],
                                rhs=oh[:, col_rel, :],
                                start=(ci == 0),
                                stop=(ci == nk - 1),
                            )
                    nT = wpool.tile([P, BPC, P], BF16, tag="nT")
                    for j in range(BPC):
                        nc.scalar.activation(
                            out=nT[:, j, :], in_=nTp[j][:, 0:P], func=AF.Copy
                        )

                    # per-block feature-major x from resident fp32 x
                    tpx = psB.tile([P, BPC, P], F32, tag="tpx", space="PSUM")
                    for j in range(BPC):
                        nc.tensor.transpose(
                            out=tpx[:, j, :pb], in_=x_res[:pb, b, j, :],
                            identity=idf_t[:pb, :pb],
                        )
                    xTb = wpool.tile([P, BPC, P], BF16, tag="xTb")
                    nc.scalar.activation(
                        out=xTb[:, :, :pb], in_=tpx[:, :, :pb], func=AF.Copy
                    )

                    h_ps = psB.tile([P, BPC, H], F32, tag="h_ps", space="PSUM")
                    for j in range(BPC):
                        nc.tensor.matmul(
                            out=h_ps[:, j, :pb], lhsT=w1a_t[:],
                            rhs=nT[:, j, :pb], start=True, stop=False,
                        )
                        nc.tensor.matmul(
                            out=h_ps[:, j, :pb], lhsT=w1b_t[:],
                            rhs=xTb[:, j, :pb], start=False, stop=True,
                        )
                    h_sb = wpool.tile([P, BPC, H], BF16, tag="h_sb")
                    nc.scalar.activation(
                        out=h_sb[:, :, :pb], in_=h_ps[:, :, :pb], func=AF.Relu,
                        bias=b1c[:],
                    )

                    imp_ps = psB.tile([P, BPC, H], F32, tag="imp_ps",
                                      space="PSUM")
                    for j in range(BPC):
                        nc.tensor.matmul(
                            out=imp_ps[:, j, :pb], lhsT=w2_t[:],
                            rhs=h_sb[:, j, :pb], start=True, stop=True,
                        )
                    imp_sb = wpool.tile([P, BPC, H], BF16, tag="imp_sb")
                    nc.vector.tensor_scalar(
                        out=imp_sb[:, :, :pb], in0=imp_ps[:, :, :pb],
                        scalar1=b2c[:], scalar2=None, op0=ALU.add,
                    )

                    outT = psC.tile([P, BPC, P], BF16, tag="outT", space="PSUM")
                    for j in range(BPC):
                        nc.tensor.transpose(
                            out=outT[:pb, j, :], in_=imp_sb[:, j, :pb],
                            identity=idb_t[:],
                        )

                    xt_sb = wpool.tile([P, BPC, H], F32, tag="xt_sb")
                    for j in range(BPC):
                        nc.scalar.activation(
                            out=xt_sb[:pb, j, :], in_=x_res[:pb, b, j, :],
                            func=AF.Copy, scale=omm[:pb, b:b + 1, j],
                        )
                    out_sb = wpool.tile([P, BPC, H], F32, tag="out_sb")
                    for j in range(BPC):
                        nc.vector.scalar_tensor_tensor(
                            out=out_sb[:pb, j, :],
                            in0=outT[:pb, j, :],
                            scalar=m_f[:pb, b:b + 1, j],
                            in1=xt_sb[:pb, j, :],
                            op0=ALU.mult,
                            op1=ALU.add,
                        )
                    nc.sync.dma_start(
                        out=y_out[:, c0:c0 + pb, :].rearrange("b p h -> p b h"),
                        in_=out_sb[:pb],
                    )

    nc.compile()
    return nc


def kernel(node_embeddings, missing_mask, edge_index, W1, b1, W2, b2, trace=False):
    global last_results
    x = np.ascontiguousarray(np.asarray(node_embeddings, np.float32))
    mask = np.ascontiguousarray(np.asarray(missing_mask, np.int32))
    B, N, H = x.shape
    assert H == P and B % NCORES == 0
    BPC = B // NCORES

    ekey = (N, B, H, hash(np.asarray(edge_index).tobytes()),
            hash(mask.tobytes()))
    if ekey not in _cache:
        keeps = [
            (mask[c * BPC:(c + 1) * BPC] != 0).any(axis=0) for c in range(NCORES)
        ]
        spec, idx_list, dl1_list, dl2_list, dl3_list = prep_pairs(
            edge_index, N, keeps
        )
        nc = _build(spec, B, N, H)
        _cache[ekey] = (nc, idx_list, dl1_list, dl2_list, dl3_list)
    nc, idx_list, dl1_list, dl2_list, dl3_list = _cache[ekey]

    iota_arr = np.broadcast_to(
        np.arange(P, dtype=np.float32), (P, P)
    ).astype(NPBF16).copy()
    idb_arr = np.eye(P, dtype=np.float32).astype(NPBF16)
    idf_arr = np.eye(P, dtype=np.float32)

    W1f = np.asarray(W1, np.float32)
    W2f = np.asarray(W2, np.float32)
    common = {
        "W1a": np.ascontiguousarray(W1f[0:H].astype(NPBF16)),
        "W1b": np.ascontiguousarray(W1f[H:2 * H].astype(NPBF16)),
        "W2b": np.ascontiguousarray(W2f.astype(NPBF16)),
        "b1": np.ascontiguousarray(np.asarray(b1, np.float32)),
        "b2": np.ascontiguousarray(np.asarray(b2, np.float32)),
        "iota": iota_arr,
        "idb": idb_arr,
        "idf": idf_arr,
    }
    in_maps = []
    for c in range(NCORES):
        m = dict(common)
        m["x"] = np.ascontiguousarray(x[c * BPC:(c + 1) * BPC])
        m["mask"] = np.ascontiguousarray(mask[c * BPC:(c + 1) * BPC])
        m["idx"] = idx_list[c]
        m["dl"] = dl1_list[c]
        m["dl2"] = dl2_list[c]
        m["dl3"] = dl3_list[c]
        in_maps.append(m)

    try:
        res = bass_utils.run_bass_kernel_spmd(
            nc, in_maps, core_ids=list(range(NCORES)), trace=trace
        )
    except ModuleNotFoundError:
        res = bass_utils.run_bass_kernel_spmd(
            nc, in_maps, core_ids=list(range(NCORES)), trace=False
        )
    last_results = res
    return np.concatenate([res.results[c]["y"] for c in range(NCORES)], axis=0)
